# revision 16
# baseline (speedup 1.0000x reference)
"""Trainium2 Bass kernel for a dense transformer block (pre-LN, causal MHA, FFN).

Sharding (v2): head-parallel attention + sequence-parallel FFN, joined by a
ReduceScatter (Ulysses-style). Core c = (batch b=c//4, head-group hg=c%4).
Each core computes LN1 over its batch's full 2048 tokens, Q/K/V projections
for its 4 heads only, causal attention for those heads with no padding waste
(query-group g only visits key tiles 0..4g+3), then a PARTIAL out-projection
using its 256 rows of Wo. A 4-core ReduceScatter (bf16, ring within the batch
group) sums the partial attention outputs and scatters by token chunk --
group-rank r receives tokens [512r, 512(r+1)), matching hg == own chunk, so
all addresses stay core-invariant (SPMD). LN2 + FFN then run on the core's
own 512 tokens exactly as in v1. A tiny warm-up ReduceScatter at kernel start
absorbs the ~50us collective setup latency; the real RS costs ~20us exposed.

Softmax: scores are computed transposed (keys on partitions) so a per-key
causal triangle is a cheap multiplicative mask and AV needs no transposes;
exp uses a constant -5 shift (no max-subtraction; scores are O(1)); the
denominator rides row DH of the AV psum via a ones column in V's lhsT.
Normalization is deferred: unnormalized AV + per-(head,token) denominators
are staged per query-group, one batched reciprocal [4,512] per group, then a
PE broadcast turns recips into a [128,512] multiplier for the 2 head-pairs.
O-proj for group g overlaps attention of group g+1.

Precision: matmul datapath bf16 x bf16 -> fp32 psum; LN statistics, softmax
denominators, residuals and the final output stay fp32; the RS sums partial
out-projections in bf16 (4-way, O(1) values).
"""

import sys
from contextlib import ExitStack
from dataclasses import dataclass

import numpy as np

if "/opt/trn_rl_repo" not in sys.path:
    sys.path.insert(0, "/opt/trn_rl_repo")

import concourse.bass as bass  # noqa: E402
import concourse.mybir as mybir  # noqa: E402
import concourse.tile as tile  # noqa: E402
from concourse.vector_clock import ScopedClock  # noqa: E402

F32 = mybir.dt.float32
F32R = mybir.dt.float32r
BF16 = mybir.dt.bfloat16
AX = mybir.AluOpType
AF = mybir.ActivationFunctionType

EXP_SHIFT = -5.0


class TC(tile.TileContext):
    """TileContext whose kernel-tail drain splits its sem waits across
    separate SP instructions -- walrus in this env rejects >2 sync waits
    on one CTRL-class instruction -- and which post-splits any multi-wait
    instruction (the S3_LW fp32 matmul struct tolerates only one sync
    wait) by hoisting extra waits onto same-engine NoOps."""

    do_split_waits = True  # disable for CoreSim (breaks its fake-update bookkeeping)

    def schedule_and_allocate(self, *a, **k):
        ret = super().schedule_and_allocate(*a, **k)
        if self.do_split_waits:
            self._split_multiwaits()
        return ret

    def _split_multiwaits(self):
        import bass_rust
        n_new = 0
        for fn in self.nc.m.functions:
            for blk in fn.blocks:
                insts = list(blk.instructions)
                out = []
                changed = False
                for inst in insts:
                    si = inst.sync_info
                    waits = list(si.on_wait) if si is not None else []
                    if len(waits) > 1:
                        for w in waits[:-1]:
                            nop = mybir.InstNoOp(
                                name=f"{inst.name}-sw{n_new}", ins=[], outs=[])
                            nop.engine = inst.engine
                            nop.sync_info = bass_rust.SyncInfo(
                                on_wait=[w], on_update=[])
                            out.append(nop)
                            n_new += 1
                        si.on_wait = [waits[-1]]
                        changed = True
                    out.append(inst)
                if changed:
                    blk.instructions = out

    def _drain_and_barrier(self, tick_clock, wait_clock):
        probe = self.nc.sync.nop(nofuse=True)
        wait_clock.add_sem_waits(probe.ins, ScopedClock({None: tick_clock.global_clock}))
        waits = list(probe.ins.sync_info.on_wait)
        assert self.sems is not None
        alloc = self.sems.allocated()
        by_name = {getattr(h, "name", k): h for k, h in alloc.items()}
        if len(waits) > 1:
            probe.ins.sync_info.on_wait = [waits[0]]
            for w in waits[1:]:
                self.nc.sync.wait_ge(by_name[w.ant_name], w.wait_value)
        self.nc.sync.drain()
        self.nc.all_engine_barrier()
        popped = self.nc._tile_sem_poison_stack.pop()
        assert popped is self._sem_poison
        self.nc.clear_and_free_semaphores(list(alloc.values()))
        self.nc.all_engine_barrier()


@dataclass(frozen=True)
class Cfg:
    P: int = 128          # partitions
    E: int = 1024         # embed dim
    H: int = 16           # total heads
    HL: int = 4           # heads per core (head-group)
    DH: int = 64          # head dim
    HID: int = 4096       # ffn hidden
    S: int = 2048         # tokens per batch (full seq, per core's batch)
    TQ: int = 512         # own tokens per core (FFN / output chunk)
    eps: float = 1e-5
    n_cores: int = 8
    use_bf16: bool = True

    @property
    def ET(self):
        return self.E // self.P      # 8 feature tiles

    @property
    def G(self):
        return self.S // self.TQ     # 4 query groups

    @property
    def NPR(self):
        return self.HL // 2          # 2 head pairs per core

    @property
    def EL(self):
        return self.HL * self.DH     # 256 local attn features

    @property
    def FB(self):
        return self.EL // self.P     # 2 local feature blocks

    @property
    def HOT(self):
        return self.HID // self.P    # 32


def f32r(ap):
    return ap.bitcast(F32R)


def build_program(cfg: Cfg, split_waits: bool = True) -> bass.Bass:
    P, E, DH, HID = cfg.P, cfg.E, cfg.DH, cfg.HID
    S, TQ, ET, G, HOT, NPR, EL, FB = (
        cfg.S, cfg.TQ, cfg.ET, cfg.G, cfg.HOT, cfg.NPR, cfg.EL, cfg.FB)
    JT = S // P          # 16 key tiles per batch
    JPG = TQ // P        # 4 key tiles per query group
    NG = TQ

    DT = BF16 if cfg.use_bf16 else F32

    def rnd(ap):
        """Matmul-operand producer/consumer wrapper for the main datapath."""
        return ap if cfg.use_bf16 else ap.bitcast(F32R)

    nc = bass.Bass("TRN2", num_devices=cfg.n_cores)

    xdev = nc.declare_dram_parameter("xdev", [P, ET, S], F32, isOutput=False)
    xq = nc.declare_dram_parameter("xq", [P, ET, TQ], F32, isOutput=False)
    # per-core head-group weight slices (host-prepared, contiguous)
    Wqc = nc.declare_dram_parameter("Wqc", [P, ET, EL], DT, isOutput=False)
    Wkc = nc.declare_dram_parameter("Wkc", [P, ET, EL], DT, isOutput=False)
    Wvc = nc.declare_dram_parameter("Wvc", [P, ET, EL], DT, isOutput=False)
    Wosl = nc.declare_dram_parameter("Wosl", [FB, P, ET, P], DT, isOutput=False)
    W1c = nc.declare_dram_parameter("W1c", [HOT, P, ET, P], DT, isOutput=False)
    W2t = nc.declare_dram_parameter("W2t", [HOT, P, E], DT, isOutput=False)
    lnw1 = nc.declare_dram_parameter("lnw1", [P, ET], F32, isOutput=False)
    lnb1 = nc.declare_dram_parameter("lnb1", [P, ET], F32, isOutput=False)
    lnw2 = nc.declare_dram_parameter("lnw2", [P, ET], F32, isOutput=False)
    lnb2 = nc.declare_dram_parameter("lnb2", [P, ET], F32, isOutput=False)
    bod = nc.declare_dram_parameter("bo", [P, ET], F32, isOutput=False)
    b1d = nc.declare_dram_parameter("b1", [P, HOT], F32, isOutput=False)
    b2d = nc.declare_dram_parameter("b2", [P, ET], F32, isOutput=False)
    outT = nc.declare_dram_parameter("outT", [P, ET, TQ], F32, isOutput=True)

    # collective buffers (internal DRAM)
    warm_in = nc.dram_tensor("warm_in", [4, 64], F32)
    warm_out = nc.dram_tensor("warm_out", [1, 64], F32)
    cc_in = nc.dram_tensor("cc_in", [G, P, ET, TQ], DT)
    cc_out = nc.dram_tensor("cc_out", [P, ET, TQ], DT)
    RGROUPS = [[0, 1, 2, 3], [4, 5, 6, 7]]

    scale = 1.0 / float(np.sqrt(DH))

    _ones_row = []  # (1, P) f32r-rounded ones, set up in the const section

    def bcast(ps_pool, tag, nparts, row):
        """Broadcast a (1, n) f32r SBUF row across nparts partitions via a
        PE outer product (ones[1,nparts].T @ row) into a PSUM tile."""
        ps_b = ps_pool.tile([nparts, row.shape[-1]], F32, tag=tag, name=f"bc_{tag}")
        nc.tensor.matmul(ps_b, f32r(_ones_row[0][:, 0:nparts]), f32r(row),
                         start=True, stop=True)
        return ps_b

    def ln_stats(rows_p, pbc_p, ps_sum, ps_sq, eps_row):
        """psum sums -> (nmean_b, rstd_b) PSUM broadcast tiles."""
        n = ps_sum.shape[-1]
        nmean = rows_p.tile([1, n], F32, tag="rows")
        nc.vector.tensor_scalar_mul(f32r(nmean), ps_sum, -1.0 / E)
        msq = rows_p.tile([1, n], F32, tag="rows")
        nc.vector.tensor_mul(msq, nmean, nmean)
        var = rows_p.tile([1, n], F32, tag="rows")
        nc.vector.scalar_tensor_tensor(
            out=var, in0=ps_sq, scalar=1.0 / E, in1=msq,
            op0=AX.mult, op1=AX.subtract)
        sq = rows_p.tile([1, n], F32, tag="rows")
        nc.scalar.activation(out=sq, in_=var, func=AF.Sqrt, bias=eps_row)
        rstd = rows_p.tile([1, n], F32, tag="rows")
        nc.vector.reciprocal(f32r(rstd), sq)
        nmean_b = bcast(pbc_p, "pbc", P, nmean)
        rstd_b = bcast(pbc_p, "pbc", P, rstd)
        return nmean_b, rstd_b

    def ln_apply(tmp_p, dst, src, nmean_b, rstd_b, w_col, b_col):
        """dst = LN(src)*w + b; intermediates in fp32, final write casts."""
        t = tmp_p.tile([P, dst.shape[-1]], F32, tag="lnt")
        nc.vector.tensor_add(t, src, nmean_b)
        nc.vector.scalar_tensor_tensor(
            out=t, in0=t, scalar=w_col, in1=rstd_b,
            op0=AX.mult, op1=AX.mult)
        nc.vector.tensor_scalar_add(rnd(dst), t, b_col)

    with TC(nc, num_cores=cfg.n_cores) as tc, \
            nc.allow_low_precision(reason="reduced-precision matmul datapath"):
        tc.do_split_waits = split_waits
        with ExitStack() as top:
            const_p = top.enter_context(tc.tile_pool(name="consts", bufs=1))
            ht_p = top.enter_context(tc.tile_pool(name="ht", bufs=1))

            ones = const_p.tile([P, 1], F32)
            nc.vector.memset(ones, 1.0)
            ones_r = const_p.tile([P, 1], F32)
            nc.vector.tensor_copy(f32r(ones_r), ones)
            ones_hb = const_p.tile([P, cfg.HL, 1], F32)
            nc.vector.memset(ones_hb, 1.0)
            ones_row = const_p.tile([1, P], F32)
            nc.vector.memset(ones_row, 1.0)
            ones_row_r = const_p.tile([1, P], F32)
            nc.vector.tensor_copy(f32r(ones_row_r), ones_row)
            _ones_row.append(ones_row_r)
            eps_row = const_p.tile([1, 1], F32)
            nc.vector.memset(eps_row, cfg.eps)
            expb = const_p.tile([P, 1], F32)
            nc.vector.memset(expb, EXP_SHIFT)
            # all-ones rows at partition bases 0 and 32 (f32r), for the
            # denominator broadcast matmuls
            ones33 = const_p.tile([33, P], F32)
            nc.vector.memset(ones33, 1.0)
            ones33_r = const_p.tile([33, P], F32)
            nc.vector.tensor_copy(f32r(ones33_r), ones33)
            LNW1 = const_p.tile([P, ET], F32)
            nc.sync.dma_start(out=LNW1, in_=lnw1[:])
            LNB1 = const_p.tile([P, ET], F32)
            nc.sync.dma_start(out=LNB1, in_=lnb1[:])
            LNW2 = const_p.tile([P, ET], F32)
            nc.sync.dma_start(out=LNW2, in_=lnw2[:])
            LNB2 = const_p.tile([P, ET], F32)
            nc.sync.dma_start(out=LNB2, in_=lnb2[:])
            BO = const_p.tile([P, ET], F32)
            nc.sync.dma_start(out=BO, in_=bod[:])
            B1 = const_p.tile([P, HOT], F32)
            nc.sync.dma_start(out=B1, in_=b1d[:])
            B2 = const_p.tile([P, ET], F32)
            nc.sync.dma_start(out=B2, in_=b2d[:])

            # causal-triangle multiplicative masks, TRI[jrel]: for query group
            # g and key tile j = 4g + jrel: mask[k_p, q] = (q >= 128*jrel + k_p)
            TRI = const_p.tile([P, JPG, TQ], DT)
            with tc.tile_pool(name="trisc", bufs=2) as tri_p:
                for jj in range(JPG):
                    tsc = tri_p.tile([P, TQ], F32, tag="trisc")
                    nc.vector.memset(tsc, 1.0)
                    nc.gpsimd.affine_select(
                        out=tsc, in_=tsc, compare_op=AX.is_ge, fill=0.0,
                        base=-jj * P, pattern=[[1, TQ]], channel_multiplier=-1)
                    nc.vector.tensor_copy(rnd(TRI[:, jj, :]), tsc)

            # warm-up collective: absorbs the collective-path setup latency
            # while LN1/QKV compute runs. Issued on gpsimd AFTER the TRI
            # affine_selects so it doesn't stall them.
            wtile = const_p.tile([4, 64], F32)
            nc.vector.memset(wtile, 1.0)
            nc.sync.dma_start(out=warm_in[:], in_=wtile)
            nc.gpsimd.collective_compute(
                "ReduceScatter", AX.add, replica_groups=RGROUPS,
                ins=[warm_in[:].opt()], outs=[warm_out[:].opt()])

            with ExitStack() as mid:
                with ExitStack() as attn_sc:
                    xn_p = attn_sc.enter_context(tc.tile_pool(name="xn", bufs=1))
                    qt_p = attn_sc.enter_context(tc.tile_pool(name="qt", bufs=1))
                    kt_p = attn_sc.enter_context(tc.tile_pool(name="kt", bufs=1))
                    va_p = attn_sc.enter_context(tc.tile_pool(name="va", bufs=1))
                    XN = xn_p.tile([P, ET, S], DT)
                    QT = qt_p.tile([P, NPR, S], DT)
                    KT = kt_p.tile([P, NPR, S], DT)
                    VA = va_p.tile([P, JT, cfg.HL, DH + 1], DT)

                    # -------- phase A: LN1 + Q/K/V projections (per group) --
                    with tc.tile_pool(name="xs", bufs=ET + 2) as xs_p, \
                         tc.tile_pool(name="xsq", bufs=3) as xsq_p, \
                         tc.tile_pool(name="lnt", bufs=3) as lnt_p, \
                         tc.tile_pool(name="rows", bufs=6) as rows_p, \
                         tc.tile_pool(name="wqkv", bufs=3) as wqkv_p, \
                         tc.tile_pool(name="pstat", bufs=3, space="PSUM") as pstat_p, \
                         tc.tile_pool(name="pbc", bufs=2, space="PSUM") as pbc_p, \
                         tc.tile_pool(name="ppv", bufs=3, space="PSUM") as ppv_p:

                        wq = wqkv_p.tile([P, ET, EL], DT, tag="wqkv", name="wq")
                        nc.sync.dma_start(out=rnd(wq), in_=rnd(Wqc[:]))
                        wk = wqkv_p.tile([P, ET, EL], DT, tag="wqkv", name="wk")
                        nc.sync.dma_start(out=rnd(wk), in_=rnd(Wkc[:]))
                        wv = wqkv_p.tile([P, ET, EL], DT, tag="wqkv", name="wv")
                        nc.sync.dma_start(out=rnd(wv), in_=rnd(Wvc[:]))

                        def ln_group(g):
                            gs = slice(g * NG, (g + 1) * NG)
                            ps_sum = pstat_p.tile([1, NG], F32, tag="pstat",
                                                  name=f"pssum{g}")
                            ps_sq = pstat_p.tile([1, NG], F32, tag="pstat",
                                                 name=f"pssq{g}")
                            xs_tiles = []
                            for et in range(ET):
                                xs = xs_p.tile([P, NG], F32, tag="xs")
                                nc.sync.dma_start(out=f32r(xs),
                                                  in_=f32r(xdev[:, et, gs]))
                                xs_tiles.append(xs)
                                xsq = xsq_p.tile([P, NG], F32, tag="xsq")
                                nc.scalar.square(out=f32r(xsq), in_=xs)
                                nc.tensor.matmul(ps_sum, f32r(ones_r), f32r(xs),
                                                 start=(et == 0), stop=(et == ET - 1))
                                nc.tensor.matmul(ps_sq, f32r(ones_r), f32r(xsq),
                                                 start=(et == 0), stop=(et == ET - 1))
                            nmean_b, rstd_b = ln_stats(rows_p, pbc_p, ps_sum, ps_sq,
                                                       eps_row)
                            for et in range(ET):
                                ln_apply(lnt_p, XN[:, et, gs], xs_tiles[et],
                                         nmean_b, rstd_b,
                                         LNW1[:, et:et + 1], LNB1[:, et:et + 1])

                        for g in range(G):
                            gs = slice(g * NG, (g + 1) * NG)
                            ln_group(g)
                            # Q and K projections for this token group
                            for (w, dstT) in ((wq, QT), (wk, KT)):
                                for pr in range(NPR):
                                    ps = ppv_p.tile([P, NG], F32, tag="ppv",
                                                    name=f"psqk{g}_{pr}")
                                    for et in range(ET):
                                        nc.tensor.matmul(
                                            ps, rnd(w[:, et, pr * P:(pr + 1) * P]),
                                            rnd(XN[:, et, gs]),
                                            start=(et == 0), stop=(et == ET - 1))
                                    nc.vector.tensor_copy(rnd(dstT[:, pr, gs]), ps)
                            # V projection, token-major, for this group's tiles
                            for tt in range(g * JPG, (g + 1) * JPG):
                                nc.vector.tensor_copy(
                                    rnd(VA[:, tt, :, DH:DH + 1]), ones_hb)
                                ps = ppv_p.tile([P, EL], F32, tag="ppv",
                                                name=f"psv{tt}")
                                for et in range(ET):
                                    nc.tensor.matmul(
                                        ps, rnd(XN[:, et, tt * P:(tt + 1) * P]),
                                        rnd(wv[:, et, :]),
                                        start=(et == 0), stop=(et == ET - 1))
                                nc.vector.tensor_copy(
                                    rnd(VA[:, tt, :, 0:DH]),
                                    ps.rearrange("p (h d) -> p h d", d=DH))

                    # -------- phase B: attention + partial out-proj + RS ----
                    wo_tiles = []
                    with tc.tile_pool(name="wo", bufs=FB) as wo_p, \
                         tc.tile_pool(name="pt", bufs=4) as pt_p, \
                         tc.tile_pool(name="avs", bufs=5) as avs_p, \
                         tc.tile_pool(name="dg", bufs=2) as dg_p, \
                         tc.tile_pool(name="rg", bufs=2) as rg_p, \
                         tc.tile_pool(name="atn", bufs=2) as atn_p, \
                         tc.tile_pool(name="oout", bufs=4) as oout_p, \
                         tc.tile_pool(name="psc", bufs=3, space="PSUM") as psc_p, \
                         tc.tile_pool(name="pav", bufs=2, space="PSUM") as pav_p, \
                         tc.tile_pool(name="pm", bufs=2, space="PSUM") as pm_p, \
                         tc.tile_pool(name="po", bufs=1, space="PSUM") as po_p:
                        for fb in range(FB):
                            wo = wo_p.tile([P, ET, P], DT, tag="wo",
                                           name=f"wo{fb}")
                            nc.sync.dma_start(out=rnd(wo), in_=rnd(Wosl[fb]))
                            wo_tiles.append(wo)

                        for g in range(G):
                            gs = slice(g * NG, (g + 1) * NG)
                            njt = (g + 1) * JPG  # causal: key tiles 0..njt-1
                            avs_tiles = []
                            for pr in range(NPR):
                                ps_avA = pav_p.tile([DH + 1, TQ], F32, tag="pav")
                                ps_avB = pav_p.tile([DH + 1, TQ], F32, tag="pav")
                                for j in range(njt):
                                    js = slice(j * P, (j + 1) * P)
                                    jrel = j - g * JPG
                                    pts = []
                                    for hh, ps_av in ((0, ps_avA), (1, ps_avB)):
                                        kb0 = hh * DH
                                        psc = psc_p.tile([P, TQ], F32, tag="psc")
                                        nc.tensor.matmul(
                                            psc, rnd(KT[kb0:kb0 + DH, pr, js]),
                                            rnd(QT[kb0:kb0 + DH, pr, gs]),
                                            start=True, stop=True)
                                        pt = pt_p.tile([P, TQ], DT, tag="pt")
                                        nc.scalar.activation(
                                            out=rnd(pt), in_=psc, func=AF.Exp,
                                            bias=expb, scale=scale)
                                        if jrel >= 0:
                                            nc.vector.tensor_mul(
                                                rnd(pt), pt, TRI[:, jrel, :])
                                        pts.append(pt)
                                    hA, hB = 2 * pr, 2 * pr + 1
                                    nc.tensor.matmul(
                                        ps_avA, rnd(VA[:, j, hA, :]),
                                        rnd(pts[0]),
                                        start=(j == 0), stop=(j == njt - 1))
                                    nc.tensor.matmul(
                                        ps_avB, rnd(VA[:, j, hB, :]),
                                        rnd(pts[1]),
                                        start=(j == 0), stop=(j == njt - 1))
                                for ps_av in (ps_avA, ps_avB):
                                    av = avs_p.tile([DH + 1, TQ], F32, tag="avs")
                                    nc.vector.tensor_copy(av, ps_av)
                                    avs_tiles.append(av)
                            # deferred softmax normalization for group g:
                            # even-head denominators at partition 0, odd at
                            # partition 32 (only bases 0/32/64 are legal)
                            D_g = dg_p.tile([33, NPR, TQ], F32, tag="dg")
                            nc.vector.memset(D_g, 1.0)
                            for h in range(cfg.HL):
                                nc.vector.tensor_copy(
                                    D_g[32 * (h % 2):32 * (h % 2) + 1, h // 2, :],
                                    avs_tiles[h][DH:DH + 1, :])
                            R_g = rg_p.tile([33, NPR, TQ], F32, tag="rg")
                            nc.vector.reciprocal(f32r(R_g), D_g)
                            ATn = atn_p.tile([P, NPR, TQ], DT, tag="atn")
                            for pr in range(NPR):
                                m0 = pm_p.tile([DH, TQ], F32, tag="pm",
                                               name=f"m0_{g}_{pr}")
                                nc.tensor.matmul(
                                    m0, f32r(ones33_r[0:1, 0:DH]),
                                    f32r(R_g[0:1, pr, :]),
                                    start=True, stop=True)
                                m1 = pm_p.tile([DH, TQ], F32, tag="pm",
                                               name=f"m1_{g}_{pr}")
                                nc.tensor.matmul(
                                    m1, f32r(ones33_r[32:33, 0:DH]),
                                    f32r(R_g[32:33, pr, :]),
                                    start=True, stop=True)
                                nc.vector.tensor_mul(
                                    rnd(ATn[0:DH, pr, :]),
                                    avs_tiles[2 * pr][0:DH, :], m0)
                                nc.vector.tensor_mul(
                                    rnd(ATn[DH:P, pr, :]),
                                    avs_tiles[2 * pr + 1][0:DH, :], m1)
                            # partial out-projection for group g -> cc_in
                            for eo in range(ET):
                                ps_o = po_p.tile([P, TQ], F32, tag="po",
                                                 name=f"pso{g}_{eo}")
                                for fb in range(FB):
                                    nc.tensor.matmul(
                                        ps_o, rnd(wo_tiles[fb][:, eo, :]),
                                        rnd(ATn[:, fb, :]),
                                        start=(fb == 0), stop=(fb == FB - 1))
                                oo = oout_p.tile([P, TQ], DT, tag="oout")
                                nc.vector.tensor_copy(rnd(oo), ps_o)
                                nc.sync.dma_start(out=rnd(cc_in[g, :, eo, :]),
                                                  in_=rnd(oo))

                        nc.gpsimd.collective_compute(
                            "ReduceScatter", AX.add, replica_groups=RGROUPS,
                            ins=[cc_in[:].opt()], outs=[cc_out[:].opt()])

                # -------- residual + LN2 --------
                HT = ht_p.tile([P, ET, TQ], F32)
                lt_p = mid.enter_context(tc.tile_pool(name="lt", bufs=1))
                rt_p = mid.enter_context(tc.tile_pool(name="rt", bufs=1))
                LT = lt_p.tile([P, ET, TQ], DT)
                RT = rt_p.tile([P, HOT, TQ], DT)
                with tc.tile_pool(name="co", bufs=1) as co_p, \
                     tc.tile_pool(name="xqs", bufs=1) as xq_p, \
                     tc.tile_pool(name="lnt2", bufs=3) as lnt2_p, \
                     tc.tile_pool(name="sq2", bufs=3) as sq2_p, \
                     tc.tile_pool(name="rows2", bufs=6) as rows2_p, \
                     tc.tile_pool(name="pstat2", bufs=2, space="PSUM") as pstat2_p, \
                     tc.tile_pool(name="pbc2", bufs=2, space="PSUM") as pbc2_p:
                    CO = co_p.tile([P, ET, TQ], DT)
                    nc.sync.dma_start(out=rnd(CO), in_=rnd(cc_out[:]))
                    XQ = xq_p.tile([P, ET, TQ], F32)
                    nc.sync.dma_start(out=XQ, in_=xq[:])
                    for eo in range(ET):
                        dst = HT[:, eo, :]
                        nc.vector.tensor_add(f32r(dst), CO[:, eo, :], XQ[:, eo, :])
                        nc.vector.tensor_scalar_add(f32r(dst), dst,
                                                    BO[:, eo:eo + 1])
                    ps_sum = pstat2_p.tile([1, TQ], F32, tag="pstat2", name="l2sum")
                    ps_sq = pstat2_p.tile([1, TQ], F32, tag="pstat2", name="l2sq")
                    for et in range(ET):
                        hsq = sq2_p.tile([P, TQ], F32, tag="sq2")
                        nc.scalar.square(out=f32r(hsq), in_=HT[:, et, :])
                        nc.tensor.matmul(ps_sum, f32r(ones_r),
                                         f32r(HT[:, et, :]),
                                         start=(et == 0), stop=(et == ET - 1))
                        nc.tensor.matmul(ps_sq, f32r(ones_r), f32r(hsq),
                                         start=(et == 0), stop=(et == ET - 1))
                    nmean_b, rstd_b = ln_stats(rows2_p, pbc2_p, ps_sum, ps_sq,
                                               eps_row)
                    for et in range(ET):
                        ln_apply(lnt2_p, LT[:, et, :], HT[:, et, :],
                                 nmean_b, rstd_b,
                                 LNW2[:, et:et + 1], LNB2[:, et:et + 1])

                # -------- FFN1 + FFN2 first half (pipelined per ho) --------
                EH = ET // 2
                with tc.tile_pool(name="w1", bufs=6) as w1_p, \
                     tc.tile_pool(name="w2a", bufs=4) as w2a_p, \
                     tc.tile_pool(name="ot", bufs=3) as ot_p, \
                     tc.tile_pool(name="pf1", bufs=3, space="PSUM") as pf1_p, \
                     tc.tile_pool(name="pf2a", bufs=EH, space="PSUM") as pf2a_p:
                    ps8a = [pf2a_p.tile([P, TQ], F32, tag="pf2a", name=f"ps8a_{i}")
                            for i in range(EH)]
                    for ho in range(HOT):
                        w1s = w1_p.tile([P, ET, P], DT, tag="w1")
                        nc.sync.dma_start(out=rnd(w1s), in_=rnd(W1c[ho]))
                        ps = pf1_p.tile([P, TQ], F32, tag="pf1", name=f"psf{ho}")
                        for et in range(ET):
                            nc.tensor.matmul(ps, rnd(w1s[:, et, :]),
                                             rnd(LT[:, et, :]),
                                             start=(et == 0), stop=(et == ET - 1))
                        nc.scalar.activation(out=rnd(RT[:, ho, :]), in_=ps,
                                             func=AF.Relu, bias=B1[:, ho:ho + 1])
                        w2a = w2a_p.tile([P, EH * P], DT, tag="w2a")
                        nc.sync.dma_start(out=rnd(w2a),
                                          in_=rnd(W2t[ho, :, 0:EH * P]))
                        for eo in range(EH):
                            nc.tensor.matmul(
                                ps8a[eo], rnd(w2a[:, eo * P:(eo + 1) * P]),
                                rnd(RT[:, ho, :]),
                                start=(ho == 0), stop=(ho == HOT - 1))
                    for eo in range(EH):
                        o = ot_p.tile([P, TQ], F32, tag="ot")
                        nc.vector.tensor_add(o, ps8a[eo], HT[:, eo, :])
                        nc.vector.tensor_scalar_add(o, o, B2[:, eo:eo + 1])
                        nc.sync.dma_start(out=outT[:, eo, :], in_=o)

                # -------- FFN2 second half --------
                with tc.tile_pool(name="w2b", bufs=4) as w2b_p, \
                     tc.tile_pool(name="ot2", bufs=3) as ot2_p, \
                     tc.tile_pool(name="pf2b", bufs=ET - EH, space="PSUM") as pf2b_p:
                    ps8b = [pf2b_p.tile([P, TQ], F32, tag="pf2b", name=f"ps8b_{i}")
                            for i in range(ET - EH)]
                    for ho in range(HOT):
                        w2b = w2b_p.tile([P, E - EH * P], DT, tag="w2b")
                        nc.sync.dma_start(out=rnd(w2b),
                                          in_=rnd(W2t[ho, :, EH * P:E]))
                        for eo in range(EH, ET):
                            nc.tensor.matmul(
                                ps8b[eo - EH], rnd(w2b[:, (eo - EH) * P:(eo - EH + 1) * P]),
                                rnd(RT[:, ho, :]),
                                start=(ho == 0), stop=(ho == HOT - 1))
                    for eo in range(EH, ET):
                        o = ot2_p.tile([P, TQ], F32, tag="ot2")
                        nc.vector.tensor_add(o, ps8b[eo - EH], HT[:, eo, :])
                        nc.vector.tensor_scalar_add(o, o, B2[:, eo:eo + 1])
                        nc.sync.dma_start(out=outT[:, eo, :], in_=o)
    return nc


# ------------------------- host side -------------------------

def _np_dt(cfg: Cfg):
    if cfg.use_bf16:
        import ml_dtypes
        return ml_dtypes.bfloat16
    return np.float32


def make_weight_inputs(cfg: Cfg, Wq, Wk, Wv, Wo, W1, W2):
    """Pre-tile weights. Head-group slices are per-core; FFN slabs shared."""
    P, E, ET, HOT, EL, FB = cfg.P, cfg.E, cfg.ET, cfg.HOT, cfg.EL, cfg.FB
    dt = _np_dt(cfg)

    def col_slice(W, hg):  # (E, E) -> [P, ET, EL] slab of head-group columns
        W = np.asarray(W, dtype=np.float32)[:, hg * EL:(hg + 1) * EL]
        # slab[p, et, j] = W[et*P+p, hg*EL+j]
        return np.ascontiguousarray(
            W.reshape(ET, P, EL).transpose(1, 0, 2).astype(dt))

    def row_slice(Wo, hg):  # (E, E) -> [FB, P, ET, P] slab of head-group rows
        W = np.asarray(Wo, dtype=np.float32)[hg * EL:(hg + 1) * EL, :]
        # slab[fb, p, eo, j] = Wo[hg*EL + fb*P + p, eo*P + j]
        return np.ascontiguousarray(
            W.reshape(FB, P, ET, P).astype(dt))

    W1 = np.asarray(W1, dtype=np.float32)
    w1c = np.ascontiguousarray(
        W1.reshape(ET, P, HOT, P).transpose(2, 1, 0, 3).astype(dt))
    W2 = np.asarray(W2, dtype=np.float32)
    w2t = np.ascontiguousarray(W2.reshape(HOT, P, E).astype(dt))

    per_hg = []
    for hg in range(4):
        per_hg.append({
            "Wqc": col_slice(Wq, hg),
            "Wkc": col_slice(Wk, hg),
            "Wvc": col_slice(Wv, hg),
            "Wosl": row_slice(Wo, hg),
        })
    return per_hg, {"W1c": w1c, "W2t": w2t}


def make_core_inputs(cfg: Cfg, core: int, per_hg, shared, x, bo, ln1_w, ln1_b,
                     ln2_w, ln2_b, b1, b2):
    P, E, S, TQ, ET, HOT = cfg.P, cfg.E, cfg.S, cfg.TQ, cfg.ET, cfg.HOT
    b, hg = core // 4, core % 4

    xT = np.asarray(x[b], dtype=np.float32).T  # (E, S)
    xdev = np.ascontiguousarray(xT.reshape(ET, P, S).transpose(1, 0, 2))
    xq = np.ascontiguousarray(xdev[:, :, hg * TQ:(hg + 1) * TQ])

    def cols(v, nt):
        return np.ascontiguousarray(
            np.asarray(v, dtype=np.float32).reshape(nt, P).T)

    m = {
        "xdev": xdev, "xq": xq,
        "lnw1": cols(ln1_w, ET), "lnb1": cols(ln1_b, ET),
        "lnw2": cols(ln2_w, ET), "lnb2": cols(ln2_b, ET),
        "bo": cols(bo, ET), "b1": cols(b1, HOT), "b2": cols(b2, ET),
    }
    m.update(per_hg[hg])
    m.update(shared)
    return m


def make_all_core_inputs(cfg: Cfg, **inputs):
    per_hg, shared = make_weight_inputs(
        cfg, inputs["Wq"], inputs["Wk"], inputs["Wv"], inputs["Wo"],
        inputs["W1"], inputs["W2"])
    rest = {k: inputs[k] for k in
            ("x", "bo", "ln1_w", "ln1_b", "ln2_w", "ln2_b", "b1", "b2")}
    return [make_core_inputs(cfg, c, per_hg, shared, **rest)
            for c in range(cfg.n_cores)]


def unshard_output(cfg: Cfg, results):
    """results: list of per-core dicts with 'outT' -> full (B, S, E)."""
    P, E, TQ, ET, S = cfg.P, cfg.E, cfg.TQ, cfg.ET, cfg.S
    n_chunks = S // TQ
    B = cfg.n_cores // n_chunks
    out = np.empty((B, S, E), dtype=np.float32)
    for core in range(cfg.n_cores):
        b, ci = core // n_chunks, core % n_chunks
        oT = results[core]["outT"]  # (P, ET, TQ)
        out[b, ci * TQ:(ci + 1) * TQ, :] = (
            oT.transpose(1, 0, 2).reshape(E, TQ).T)
    return out


_CACHE = {}


def _get_program(cfg: Cfg) -> bass.Bass:
    if cfg not in _CACHE:
        _CACHE[cfg] = build_program(cfg)
    return _CACHE[cfg]


def kernel(**inputs) -> np.ndarray:
    from concourse.bass_utils import run_bass_kernel_spmd
    cfg = Cfg()
    nc = _get_program(cfg)
    in_maps = make_all_core_inputs(cfg, **inputs)
    res = run_bass_kernel_spmd(nc, in_maps, list(range(cfg.n_cores)))
    return unshard_output(cfg, res.results)
